# revision 4
# baseline (speedup 1.0000x reference)
"""Trainium2 Bass kernel for nn_AttentionBlock (B=2, T=4096, C=512, H=8 causal
attention, fused qkv projection), SPMD across 8 NeuronCores.

Sharding: core c = (batch b = c//4, head-pair g = c%4). Data parallel on B,
tensor parallel splitting the 8 heads (2 per core) and the qkv projection
columns. Each core computes its own T x T score slabs tile-by-tile (flash
style: scores stay in PSUM/SBUF, never hit HBM).

Per-core dataflow (all matmul operands bf16, accumulation fp32):
  qT/kT = w_s^T @ x^T (PE, K=512 in 4 chunks; bias on DVE).  V is produced in
  natural [keys, dims] layout directly (lhsT = xT chunk, rhs = w_v chunk),
  with the V bias folded into v_sb via sum P (v+b) / sum P = sum P v / sum P + b,
  and augmented with a ones column so the softmax denominator rides the PV
  matmul for free.
  Per 512-wide query block I: S^T tiles [128 keys, 2 heads, 512 queries] in
  one f32 PSUM tile (2 banks); the two heads' S matmuls run concurrently on
  the PE via row tiling (K=64 each, rows 0-63 / 64-127).  ONE heads-merged
  exp per (I, J) on ScalarE (scale=1/8 fused, bf16 out) - ScalarE activation
  instruction count is the wall here, so FD=1024 per ACT.  Post-exp 0/1 band
  mask on the diagonal via DVE.  P^T @ V accumulated into [65, 512] PSUM per
  head (row 64 = denominator).  Normalize: PE transpose back (f32), DVE
  reciprocal * scale, one [128,128] row-contiguous DMA per 128 queries.
"""
from contextlib import ExitStack

import numpy as np
import ml_dtypes

import concourse.bass as bass
import concourse.mybir as mybir
from concourse import bacc
from concourse.tile import TileContext
from concourse.masks import make_identity

F32 = mybir.dt.float32
BF16 = mybir.dt.bfloat16
Exp = mybir.ActivationFunctionType.Exp

B = 2
T = 4096
C = 512
H = 8
HD = 64
NCK = 4          # contraction chunks of 128
TB = 512         # query tile (psum free dim)
JB = 128         # key block (partitions)
SCALE = 0.125    # 1/sqrt(HD)


def _build(pv_lag=4, pt_bufs=12, fin_bufs=6, ob_bufs=3, st_bufs=3):
    nI = T // TB
    nT128 = T // JB
    NSUB = TB // JB

    nc = bacc.Bacc("TRN2", target_bir_lowering=False, debug=False)
    xT_d = nc.dram_tensor("xT", [C, T], BF16, kind="ExternalInput")
    w_d = nc.dram_tensor("w", [NCK, 128, 384], BF16, kind="ExternalInput")
    b_d = nc.dram_tensor("bias", [3, 128, 1], F32, kind="ExternalInput")
    bv_d = nc.dram_tensor("bvrow", [1, 128], BF16, kind="ExternalInput")
    out_d = nc.dram_tensor("out", [T, 128], BF16, kind="ExternalOutput")

    with TileContext(nc) as tc, ExitStack() as stk:
        pp = stk.enter_context(tc.tile_pool(name="persist", bufs=1))
        st_ps = stk.enter_context(
            tc.tile_pool(name="st_ps", bufs=st_bufs, space="PSUM"))
        ot_ps = stk.enter_context(tc.tile_pool(name="ot_ps", bufs=1, space="PSUM"))
        pt_pool = stk.enter_context(tc.tile_pool(name="pt_pool", bufs=pt_bufs))
        ob_pool = stk.enter_context(tc.tile_pool(name="ob_pool", bufs=ob_bufs))
        fin_pool = stk.enter_context(tc.tile_pool(name="fin_pool", bufs=fin_bufs))

        xT_sb = pp.tile([128, NCK, T], BF16)
        w_sb = pp.tile([128, NCK, 384], BF16)
        bias_sb = pp.tile([128, 3], F32)
        bvrow_sb = pp.tile([1, 128], BF16)
        bias_bc = pp.tile([128, 128], BF16)
        qT_sb = pp.tile([128, T], BF16)
        kT_sb = pp.tile([128, T], BF16)
        v_sb = [pp.tile([128, nT128, 65], BF16, tag=f"v{h}", name=f"v{h}")
                for h in (0, 1)]
        ident_b = pp.tile([128, 128], BF16)
        ident_f = pp.tile([128, 128], F32)
        ones_b = pp.tile([128, 1], BF16)
        ones_row = pp.tile([1, 128], BF16)
        bmask = pp.tile([128, 128], BF16)

        make_identity(nc, ident_f[:])
        nc.vector.tensor_copy(ident_b[:], ident_f[:])
        nc.vector.memset(ones_b[:], 1.0)
        nc.vector.memset(ones_row[:], 1.0)
        nc.vector.memset(bmask[:], 1.0)
        nc.gpsimd.affine_select(
            out=bmask[:], in_=bmask[:],
            compare_op=mybir.AluOpType.is_ge,
            fill=0.0,
            base=0,
            pattern=[[1, 128]],
            channel_multiplier=-1,
        )
        for h in (0, 1):
            nc.vector.tensor_copy(
                v_sb[h][:, :, 64], ones_b[:].broadcast_to([128, nT128])
            )
        # warmup: trigger the exp table load while DMAs stream
        warm = fin_pool.tile([128, 1], BF16, tag="warm", name="warm")
        nc.scalar.activation(warm[:], ones_b[:], Exp, scale=0.0)

        nc.sync.dma_start(w_sb[:], w_d[:].rearrange("a b c -> b a c"))
        for s in range(2):
            nc.gpsimd.dma_start(bias_sb[:, s:s + 1], b_d[s])
        nc.gpsimd.dma_start(bvrow_sb[:], bv_d[:])
        bounds = sorted(set([0, min(TB, T), min(2 * TB, T), T]))
        for lo2, hi2 in zip(bounds, bounds[1:]):
            if hi2 > lo2:
                for ck in range(NCK):
                    nc.sync.dma_start(
                        xT_sb[:, ck, lo2:hi2],
                        xT_d[ck * 128:(ck + 1) * 128, lo2:hi2],
                    )

        # broadcast b_v row across partitions: bias_bc[j, d] = b_v[d]
        bc_ps = st_ps.tile([128, 2, TB], F32, tag="st", name="st")
        nc.tensor.matmul(bc_ps[:, 0, 0:128], ones_row[:], bvrow_sb[:],
                         start=True, stop=True)
        nc.vector.tensor_copy(bias_bc[:], bc_ps[:, 0, 0:128])

        dests = [qT_sb, kT_sb]

        def proj_s(tb, s):
            ps = st_ps.tile([128, 2, TB], F32, tag="st", name="st")
            for ck in range(NCK):
                nc.tensor.matmul(
                    ps[:, 0, :],
                    w_sb[:, ck, s * 128:(s + 1) * 128],
                    xT_sb[:, ck, tb * TB:(tb + 1) * TB],
                    start=(ck == 0),
                    stop=(ck == NCK - 1),
                )
            nc.vector.tensor_scalar_add(
                dests[s][:, tb * TB:(tb + 1) * TB], ps[:, 0, :],
                bias_sb[:, s:s + 1],
            )

        def proj_v(tb, sub):
            t128 = tb * NSUB + sub
            vn = st_ps.tile([128, 2, TB], F32, tag="st", name="st")
            for ck in range(NCK):
                nc.tensor.matmul(
                    vn[:, 0, 0:128],
                    xT_sb[:, ck, t128 * JB:(t128 + 1) * JB],
                    w_sb[:, ck, 256:384],
                    start=(ck == 0),
                    stop=(ck == NCK - 1),
                )
            for h in (0, 1):
                nc.vector.tensor_add(
                    v_sb[h][:, t128, 0:64],
                    vn[:, 0, h * 64:h * 64 + 64],
                    bias_bc[:, h * 64:h * 64 + 64],
                )

        def proj_steps(tb):
            steps = [lambda s=s: proj_s(tb, s) for s in (0, 1)]
            steps += [lambda sub=sub: proj_v(tb, sub) for sub in range(NSUB)]
            return steps

        ots_of = {}

        def get_ots(I):
            if I not in ots_of:
                ots_of[I] = [
                    ot_ps.tile([65, TB], F32, tag=f"ot{h}", name=f"ot{h}")
                    for h in (0, 1)
                ]
            return ots_of[I]

        def flush_pv(prev):
            pI, pJ, pptb, p, _pnorm = prev
            jmax = NSUB * (pI + 1)
            ots = get_ots(pI)
            lo = JB * p if p >= 1 else 0
            for h in (0, 1):
                nc.tensor.matmul(
                    ots[h][:, lo:],
                    v_sb[h][:, pJ, :],
                    pptb[:, h, lo:],
                    start=(pJ == 0), stop=(pJ == jmax - 1),
                )

        def normalize(I):
            ots = ots_of.pop(I)
            obs = []
            for h in (0, 1):
                ob = ob_pool.tile([65, TB], F32, tag="ob", name="ob")
                nc.vector.tensor_copy(ob[:], ots[h][:])
                obs.append(ob)
            tp = st_ps.tile([128, 2, TB], F32, tag="st", name="st")
            for cp in range(NSUB):
                ofin = fin_pool.tile([128, 128], BF16, tag="ofin")
                for h in (0, 1):
                    dst = tp[:, h, cp * 128:cp * 128 + 65]
                    nc.tensor.transpose(
                        dst, obs[h][:, cp * 128:(cp + 1) * 128],
                        ident_f[0:65, 0:65],
                    )
                    rec = fin_pool.tile([128, 1], F32, tag="rec")
                    nc.vector.reciprocal(
                        rec[:], tp[:, h, cp * 128 + 64:cp * 128 + 65])
                    nc.vector.tensor_scalar_mul(
                        ofin[:, h * 64:(h + 1) * 64],
                        tp[:, h, cp * 128:cp * 128 + 64], rec[:])
                nc.sync.dma_start(
                    out_d[I * TB + cp * 128: I * TB + (cp + 1) * 128, :],
                    ofin[:],
                )

        groups = []
        for I in range(nI):
            njs = NSUB * (I + 1)
            proj_at = max(1, njs // 2)
            for J in range(njs):
                groups.append((I, J, J == proj_at and I + 1 < nI,
                               J == njs - 1))

        pending = []
        pq = []
        proj_s(0, 0)
        proj_s(0, 1)
        pq.extend(lambda sub=sub: proj_v(0, sub) for sub in range(NSUB))

        def drain_one():
            ent = pending.pop(0)
            flush_pv(ent)
            if ent[4]:
                normalize(ent[0])

        for (I, J, do_proj, last_of_I) in groups:
            if len(pending) > pv_lag:
                drain_one()
            if do_proj:
                pq.extend(proj_steps(I + 1))
            if pq:
                pq.pop(0)()
            stb = st_ps.tile([128, 2, TB], F32, tag="st", name="st")
            p = J - NSUB * I
            lo = JB * p if p >= 1 else 0
            for h in (0, 1):
                nc.tensor.matmul(
                    stb[:, h, lo:],
                    kT_sb[h * 64:(h + 1) * 64, J * JB:(J + 1) * JB],
                    qT_sb[h * 64:(h + 1) * 64, I * TB + lo:(I + 1) * TB],
                    start=True, stop=True,
                )
            ptb = pt_pool.tile([128, 2, TB], BF16)
            nc.scalar.activation(
                ptb[:, :, lo:], stb[:, :, lo:], Exp, scale=SCALE
            )
            if p >= 0:
                for h in (0, 1):
                    nc.vector.tensor_mul(
                        ptb[:, h, lo:lo + JB],
                        ptb[:, h, lo:lo + JB],
                        bmask[:],
                    )
            pending.append((I, J, ptb, p, last_of_I))
        while pending:
            drain_one()
    nc.compile()
    return nc


def _core_inputs(x_b, w_qkv, b_qkv, g):
    cols = np.concatenate([
        np.arange(128 * g, 128 * (g + 1)),
        512 + np.arange(128 * g, 128 * (g + 1)),
        1024 + np.arange(128 * g, 128 * (g + 1)),
    ])
    w4 = np.ascontiguousarray(
        w_qkv[:, cols].reshape(NCK, 128, 384).astype(ml_dtypes.bfloat16))
    bias = np.ascontiguousarray(
        b_qkv[cols].reshape(3, 128, 1).astype(np.float32))
    bvrow = np.ascontiguousarray(
        b_qkv[1024 + 128 * g:1024 + 128 * (g + 1)].reshape(1, 128)
    ).astype(ml_dtypes.bfloat16)
    xT = np.ascontiguousarray(x_b.T.astype(ml_dtypes.bfloat16))
    return {"xT": xT, "w": w4, "bias": bias, "bvrow": bvrow}


class _Runner:
    """Build the jitted SPMD callable once; reuse across kernel() calls."""

    def __init__(self, nc, n_cores=8):
        import jax
        from jax.sharding import Mesh, PartitionSpec, NamedSharding
        from jax.experimental.shard_map import shard_map
        from concourse.bass2jax import (
            _bass_exec_p, install_neuronx_cc_hook, partition_id_tensor,
        )
        install_neuronx_cc_hook()
        self.jax = jax
        partition_name = (
            nc.partition_id_tensor.name if nc.partition_id_tensor else None
        )
        in_names, out_names, out_avals, zero_shapes = [], [], [], []
        for alloc in nc.m.functions[0].allocations:
            if not isinstance(alloc, mybir.MemoryLocationSet):
                continue
            name = alloc.memorylocations[0].name
            if alloc.kind == "ExternalInput":
                if name != partition_name:
                    in_names.append(name)
            elif alloc.kind == "ExternalOutput":
                shape = tuple(alloc.tensor_shape)
                dtype = mybir.dt.np(alloc.dtype)
                out_names.append(name)
                out_avals.append(jax.core.ShapedArray(shape, dtype))
                zero_shapes.append((shape, dtype))
        self.in_names = in_names
        self.out_names = out_names
        self.out_avals = out_avals
        self.n_cores = n_cores
        all_in = list(in_names) + list(out_names)
        if partition_name is not None:
            all_in.append(partition_name)

        def _body(*args):
            operands = list(args)
            if partition_name is not None:
                operands.append(partition_id_tensor())
            outs = _bass_exec_p.bind(
                *operands,
                out_avals=tuple(out_avals),
                in_names=tuple(all_in),
                out_names=tuple(out_names),
                lowering_input_output_aliases=(),
                sim_require_finite=True,
                sim_require_nnan=True,
                nc=nc,
            )
            return tuple(outs)

        devices = jax.devices()[:n_cores]
        mesh = Mesh(np.asarray(devices), ("core",))
        n_params = len(in_names)
        in_specs = (PartitionSpec("core"),) * (n_params + len(out_names))
        out_specs = (PartitionSpec("core"),) * len(out_names)
        self.fn = jax.jit(
            shard_map(_body, mesh=mesh, in_specs=in_specs,
                      out_specs=out_specs, check_rep=False),
            keep_unused=True,
        )
        self.sharding = NamedSharding(mesh, PartitionSpec("core"))
        self.zero_shapes = zero_shapes

    def run(self, in_maps):
        jax = self.jax
        per_core = [[np.asarray(m[n]) for n in self.in_names] for m in in_maps]
        dev_in = [
            jax.device_put(
                np.concatenate([per_core[c][i] for c in range(self.n_cores)], 0),
                self.sharding,
            )
            for i in range(len(self.in_names))
        ]
        dev_zero = [
            jax.device_put(
                np.zeros((self.n_cores * s[0], *s[1:]), d), self.sharding
            )
            for (s, d) in self.zero_shapes
        ]
        outs = self.fn(*dev_in, *dev_zero)
        jax.block_until_ready(outs)
        res = []
        for c in range(self.n_cores):
            d = {}
            for i, name in enumerate(self.out_names):
                full = np.asarray(outs[i])
                d[name] = full.reshape(self.n_cores, *self.out_avals[i].shape)[c]
            res.append(d)
        return res


_CACHE = {}


def kernel(x, w_qkv, b_qkv):
    x = np.asarray(x, np.float32)
    w_qkv = np.asarray(w_qkv, np.float32)
    b_qkv = np.asarray(b_qkv, np.float32)
    in_maps = [
        _core_inputs(x[c // 4], w_qkv, b_qkv, c % 4) for c in range(8)
    ]
    last_err = None
    for _attempt in range(2):
        try:
            if "runner" not in _CACHE:
                _CACHE["runner"] = _Runner(_build())
            res = _CACHE["runner"].run(in_maps)
            break
        except Exception as e:  # transient NRT device errors: rebuild + retry
            last_err = e
            _CACHE.pop("runner", None)
    else:
        raise last_err
    out = np.empty((B, T, C), np.float32)
    for c in range(8):
        b, g = c // 4, c % 4
        out[b, :, 128 * g:128 * (g + 1)] = res[c]["out"].astype(np.float32)
    return out
